# revision 46
# baseline (speedup 1.0000x reference)
import sys

sys.path.insert(0, "/opt/trn_rl_repo")

import numpy as np
import ml_dtypes

import concourse.bacc as bacc
import concourse.tile as tile
from concourse import mybir
from concourse.bass_utils import run_bass_kernel_spmd

F32 = mybir.dt.float32
BF16 = mybir.dt.bfloat16
Exp = mybir.ActivationFunctionType.Exp

B, T, E = 2, 2048, 1024
H, D = 16, 64
NCORES = 8
HPC = 4
QC = HPC * D
P = 128
KC = E // P
NT = T // P

_PROG = None


def _build():
    nc = bacc.Bacc("TRN2", target_bir_lowering=False, debug=False)

    xt_d = nc.dram_tensor("xt", [E, T], BF16, kind="ExternalInput")
    wqk_d = nc.dram_tensor("wqk", [E, 2 * QC], BF16, kind="ExternalInput")
    wv_d = nc.dram_tensor("wv", [E, QC], BF16, kind="ExternalInput")
    wp_d = nc.dram_tensor("wp", [QC, E], BF16, kind="ExternalInput")
    mk_d = nc.dram_tensor("masks", [P, 2, 256], BF16, kind="ExternalInput")
    out_d = nc.dram_tensor("out", [T, E], BF16, kind="ExternalOutput")

    with tile.TileContext(nc) as tc:
        with (
            tc.tile_pool(name="persist", bufs=1) as persist,
            tc.tile_pool(name="pt", bufs=4) as ptp,
            tc.tile_pool(name="small", bufs=2) as small,
            tc.tile_pool(name="stage", bufs=4) as stg,
            tc.tile_pool(name="spsum", bufs=2, space="PSUM") as spool,
            tc.tile_pool(name="ppsum", bufs=2, space="PSUM") as ppool,
            tc.tile_pool(name="fpsum", bufs=2, space="PSUM") as fpool,
        ):
            qk_sb = [persist.tile([P, T], BF16, name=f"qk{m}") for m in range(4)]
            at_sb = [persist.tile([P, T], BF16, name=f"at{c}") for c in range(2)]
            v_sb = [persist.tile([P, HPC, D + 1], BF16, name=f"v{t}") for t in range(NT)]
            mask_sb = persist.tile([P, 2, 256], BF16, name="masks")
            wp_all = persist.tile([P, 2, E], BF16, name="wp_all")
            xt_all = persist.tile([P, KC, T], BF16, name="xt_all")
            wqk_p = [persist.tile([P, KC, QC], BF16, name=f"wqk_p{i}") for i in range(2)]
            wv_all = persist.tile([P, KC, QC], BF16, name="wv_all")

            xt_r = xt_d.rearrange("(c p) t -> p c t", c=KC)
            wqk_r = wqk_d.rearrange("(c p) w -> p c w", c=KC)
            wv_r = wv_d.rearrange("(c p) w -> p c w", c=KC)
            wp_r = wp_d.rearrange("(c p) e -> p c e", c=2)
            nc.sync.dma_start(out=xt_all[:, :, 0:256], in_=xt_r[:, :, 0:256])
            nc.sync.dma_start(out=wqk_p[0], in_=wqk_r[:, :, 0:256])
            nc.sync.dma_start(out=xt_all[:, :, 256:512], in_=xt_r[:, :, 256:512])
            nc.sync.dma_start(out=mask_sb, in_=mk_d[:])
            nc.sync.dma_start(out=wv_all, in_=wv_r)
            nc.sync.dma_start(out=xt_all[:, :, 512:1024], in_=xt_r[:, :, 512:1024])
            nc.sync.dma_start(out=wqk_p[1], in_=wqk_r[:, :, 256:512])
            nc.sync.dma_start(out=xt_all[:, :, 1024:1536], in_=xt_r[:, :, 1024:1536])
            nc.sync.dma_start(out=xt_all[:, :, 1536:2048], in_=xt_r[:, :, 1536:2048])
            nc.sync.dma_start(out=wp_all, in_=wp_r)

            ones4 = persist.tile([P, HPC, 1], F32, name="ones4")
            nc.vector.memset(ones4, 1.0)
            for t in range(NT):
                nc.vector.tensor_copy(v_sb[t][:, :, D : D + 1], ones4)

            st_tiles = {}

            def qk_chain(mb, u, nl, col0=0, width=512, act_copy=False):
                fp = fpool.tile([P, 512], F32, name="fq", tag="fp")
                base = u * 1024 + nl * 512 + col0
                for c in range(KC):
                    def mm(c=c, fp=fp):
                        nc.tensor.matmul(
                            fp[:, 0:width],
                            lhsT=wqk_p[mb // 2][:, c, (mb % 2) * P : (mb % 2) * P + P],
                            rhs=xt_all[:, c, base : base + width],
                            start=(c == 0),
                            stop=(c == KC - 1),
                        )
                    yield width, mm
                if act_copy:
                    nc.scalar.copy(qk_sb[mb][:, base : base + width], fp[:, 0:width])
                else:
                    nc.vector.tensor_copy(
                        qk_sb[mb][:, base : base + width], fp[:, 0:width]
                    )

            def v_chain(t, act_copy=False):
                fp = fpool.tile([P, 512], F32, name="fv", tag="fp")
                for c in range(KC):
                    def mm(c=c, fp=fp):
                        nc.tensor.matmul(
                            fp[:, 0:QC],
                            lhsT=xt_all[:, c, t * P : (t + 1) * P],
                            rhs=wv_all[:, c, :],
                            start=(c == 0),
                            stop=(c == KC - 1),
                        )
                    yield 256, mm
                dst = v_sb[t][:, :, 0:D]
                srcv = fp[:, 0:QC].rearrange("p (h d) -> p h d", h=HPC)
                if act_copy:
                    nc.scalar.copy(dst, srcv)
                else:
                    nc.vector.tensor_copy(dst, srcv)

            def out_chain(t, nl):
                fp = fpool.tile([P, 512], F32, name="fo", tag="fp")
                for c in range(2):
                    def mm(c=c, fp=fp):
                        nc.tensor.matmul(
                            fp,
                            lhsT=at_sb[c][:, t * P : (t + 1) * P],
                            rhs=wp_all[:, c, nl * 512 : (nl + 1) * 512],
                            start=(c == 0),
                            stop=(c == 1),
                        )
                    yield 512, mm
                tp = t // 2
                if tp not in st_tiles:
                    st_tiles[tp] = stg.tile([P, 2, E], BF16, name=f"st{tp}", tag="st")
                st = st_tiles[tp]
                nc.vector.tensor_copy(st[:, t % 2, nl * 512 : (nl + 1) * 512], fp)
                if nl == 1 and t % 2 == 1:
                    nc.sync.dma_start(
                        out=out_d[(t - 1) * P : (t + 1) * P, :].rearrange(
                            "(s p) e -> p s e", s=2
                        ),
                        in_=st,
                    )

            specs = []
            specs += [("qk000a", qk_chain(0, 0, 0, 0, 256)),
                      ("qk100a", qk_chain(1, 0, 0, 0, 256)),
                      ("qk000b", qk_chain(0, 0, 0, 256, 256)),
                      ("qk100b", qk_chain(1, 0, 0, 256, 256))]
            specs += [("v0", v_chain(0, act_copy=True)), ("v1", v_chain(1, act_copy=True)), ("v2", v_chain(2)), ("v3", v_chain(3))]
            specs += [(f"qk{mb}01", qk_chain(mb, 0, 1, act_copy=True)) for mb in (0, 1)]
            specs += [(f"v{t}", v_chain(t)) for t in range(4, 8)]
            specs += [(f"qk{mb}00", qk_chain(mb, 0, 0, act_copy=True)) for mb in (2, 3)]
            specs += [(f"qk{mb}01", qk_chain(mb, 0, 1, act_copy=True)) for mb in (2, 3)]
            specs += [("qk010", qk_chain(0, 1, 0, act_copy=True)), ("qk110", qk_chain(1, 1, 0, act_copy=True))]
            specs += [(f"out{t}.{nl}", out_chain(t, nl)) for t in (0, 1) for nl in (0, 1)]
            specs += [("v8", v_chain(8, act_copy=True)), ("v9", v_chain(9, act_copy=True))]
            specs += [("qk210", qk_chain(2, 1, 0)), ("qk310", qk_chain(3, 1, 0))]
            specs += [(f"out{t}.{nl}", out_chain(t, nl)) for t in (2, 3) for nl in (0, 1)]
            specs += [("v10", v_chain(10)), ("v11", v_chain(11))]
            specs += [("qk011", qk_chain(0, 1, 1, act_copy=True)), ("qk111", qk_chain(1, 1, 1, act_copy=True))]
            specs += [(f"out{t}.{nl}", out_chain(t, nl)) for t in (4, 5) for nl in (0, 1)]
            specs += [("v12", v_chain(12)), ("v13", v_chain(13))]
            specs += [("qk211", qk_chain(2, 1, 1)), ("qk311", qk_chain(3, 1, 1))]
            specs += [(f"out{t}.{nl}", out_chain(t, nl)) for t in (6, 7) for nl in (0, 1)]
            specs += [("v14", v_chain(14)), ("v15", v_chain(15))]
            specs += [(f"out{t}.{nl}", out_chain(t, nl)) for t in range(8, 16) for nl in (0, 1)]

            names = [n for n, _ in specs]
            assert len(set(names)) == len(names)
            name_idx = {n: i for i, n in enumerate(names)}
            qpos = [0]
            done_names = set()

            def pull(budget_cols, limit=None):
                lim = len(specs) - 1 if limit is None else name_idx[limit]
                spent = 0
                while qpos[0] <= lim and spent < budget_cols:
                    name, g = specs[qpos[0]]
                    try:
                        cost, emit = next(g)
                    except StopIteration:
                        done_names.add(name)
                        qpos[0] += 1
                        continue
                    emit()
                    spent += cost
                return spent

            def drain_through(name):
                while name not in done_names:
                    if qpos[0] >= len(specs):
                        raise RuntimeError(f"drain_through({name}): queue empty")
                    pull(1 << 30, limit=name)

            pend = [None]

            def flush_pend():
                if pend[0] is None:
                    return
                ctx, g, pt, ws = pend[0]
                pend[0] = None
                pso, po, ilast = ctx["pso"], ctx["po"], ctx["ilast"]
                for s in range(2):
                    i = 2 * g + s
                    for hh in range(2):
                        nc.tensor.matmul(
                            pso[:, hh, ws[s] : 256],
                            lhsT=v_sb[i][:, 2 * po + hh, :],
                            rhs=pt[:, hh, s, ws[s] : 256],
                            start=(i == 0 and hh == 0),
                            stop=(i == ilast and hh == 1),
                        )
                if g * 2 + 1 == ilast:
                    q0 = ctx["q0"]
                    den = small.tile([1, 2, 256], F32, name="den", tag="den")
                    nc.vector.tensor_copy(den, pso[D : D + 1, :, 0:256])
                    rec = small.tile([1, 2, 256], F32, name="rec", tag="rec")
                    nc.vector.reciprocal_approx_fast(out=rec, in_=den)
                    rb = small.tile([D, 2, 256], F32, name="rb", tag="rb")
                    nc.gpsimd.partition_broadcast(rb, rec)
                    for hh in range(2):
                        nc.vector.tensor_mul(
                            at_sb[po][64 * hh : 64 * hh + 64, q0 : q0 + 256],
                            pso[0:D, hh, 0:256],
                            rb[:, hh, :],
                        )

            def attn_chain(po, p, jj2, deadlines=None, budget=1024, fence=None):
                deadlines = deadlines or {}
                Qh, Kh = qk_sb[2 * po], qk_sb[2 * po + 1]
                q0 = p * 1024 + jj2 * 256
                ilast = 8 * p + 2 * jj2 + 1
                ngran = (ilast + 1) // 2
                pso = ppool.tile([D + 1, 2, 256], F32, name="pso", tag="pso")
                ctx = {"pso": pso, "po": po, "ilast": ilast, "q0": q0}

                for g in range(ngran):
                    for nm in deadlines.get(g, ()):
                        drain_through(nm)
                    ps = spool.tile([P, 2, 2, 256], F32, name="ps", tag="ps")
                    pt = ptp.tile([P, 2, 2, 256], BF16, name="pt")
                    ws = []
                    for s in range(2):
                        i = 2 * g + s
                        m = i - 8 * p - 2 * jj2
                        w = max(0, 128 * m)
                        ws.append(w)
                        for hh in range(2):
                            nc.tensor.matmul(
                                ps[:, hh, s, w:256],
                                lhsT=Kh[64 * hh : 64 * hh + 64, i * P : (i + 1) * P],
                                rhs=Qh[64 * hh : 64 * hh + 64, q0 + w : q0 + 256],
                                start=(s == 0),
                                stop=(s == 1),
                            )
                    if ws[1] == 0:
                        nc.scalar.activation(pt, ps, Exp, scale=0.125)
                    else:
                        nc.scalar.activation(pt[:, :, 0], ps[:, :, 0], Exp, scale=0.125)
                        nc.scalar.activation(
                            pt[:, :, 1, 128:256], ps[:, :, 1, 128:256],
                            Exp, scale=0.125,
                        )
                    flush_pend()
                    pull(budget, limit=fence)
                    for s in range(2):
                        m = 2 * g + s - 8 * p - 2 * jj2
                        if m >= 0:
                            for hh in range(2):
                                nc.vector.tensor_mul(
                                    pt[:, hh, s, ws[s] : 256],
                                    pt[:, hh, s, ws[s] : 256],
                                    mask_sb[:, m, ws[s] : 256],
                                )
                    pend[0] = (ctx, g, pt, ws)
                for nm in deadlines.get(ngran, ()):
                    drain_through(nm)

            drain_through("qk100a")

            attn_chain(0, 0, 0, fence="qk111", budget=1280,
                       deadlines={1: ["qk100b", "v0", "v1"]})
            attn_chain(0, 0, 1, fence="qk111", budget=1280,
                       deadlines={1: ["qk001"], 2: ["v2", "v3"]})
            attn_chain(0, 0, 2, fence="qk111", budget=1280,
                       deadlines={1: ["qk101"], 3: ["v4", "v5"]})
            attn_chain(0, 0, 3, fence="qk111", budget=1280,
                       deadlines={3: ["qk200", "qk300"], 4: ["v6", "v7"]})
            attn_chain(1, 0, 0, fence="qk111", budget=1280, deadlines={})
            attn_chain(1, 0, 1, fence="out1.1", budget=1280,
                       deadlines={1: ["qk201"]})
            attn_chain(1, 0, 2, fence="out3.1", budget=1280,
                       deadlines={1: ["qk301"]})
            attn_chain(1, 0, 3, fence="out3.1", budget=1280,
                       deadlines={3: ["qk010"]})
            attn_chain(0, 1, 0, fence="out3.1",
                       deadlines={3: ["qk110"], 4: ["qk210"], 5: ["v8", "v9"]})
            attn_chain(1, 1, 0, fence="out5.1",
                       deadlines={3: ["qk310"], 4: ["qk011"]})
            attn_chain(0, 1, 1, fence="out9.1",
                       deadlines={5: ["qk111"], 6: ["v10", "v11"]})
            attn_chain(1, 1, 1, fence="out9.1",
                       deadlines={5: ["qk211"]})
            attn_chain(0, 1, 2, fence="out11.1",
                       deadlines={6: ["qk311"], 7: ["v12", "v13"]})
            attn_chain(1, 1, 2, fence="out11.1", deadlines={})
            attn_chain(0, 1, 3, fence="out13.1", budget=2048,
                       deadlines={8: ["v14", "v15"]})
            attn_chain(1, 1, 3, fence="out13.1", budget=2048, deadlines={})
            flush_pend()

            while qpos[0] < len(specs):
                pull(1 << 30)

    nc.compile()
    return nc


def _get_prog():
    global _PROG
    if _PROG is None:
        _PROG = _build()
    return _PROG


def _masks_np():
    kk = np.arange(P)[:, None]
    qq = np.arange(256)[None, :]
    return np.stack(
        [((128 * m + kk) <= qq) for m in range(2)], axis=1
    ).astype(ml_dtypes.bfloat16)


def _bf(a):
    return np.ascontiguousarray(a).astype(ml_dtypes.bfloat16)


def _shard(x, w_qkv, w_proj):
    masks = _masks_np()
    in_maps = []
    for core in range(NCORES):
        b, g = core // HPC, core % HPC
        c0 = g * QC
        wq = w_qkv[:, c0 : c0 + QC]
        wk = w_qkv[:, E + c0 : E + c0 + QC]
        wqk = np.concatenate(
            [wq[:, 0:128], wk[:, 0:128], wq[:, 128:256], wk[:, 128:256]], axis=1
        )
        in_maps.append(
            {
                "xt": _bf(x[b].T),
                "wqk": _bf(wqk),
                "wv": _bf(w_qkv[:, 2 * E + c0 : 2 * E + c0 + QC]),
                "wp": _bf(w_proj[c0 : c0 + QC, :]),
                "masks": masks,
            }
        )
    return in_maps


def _run(inputs, **kwargs):
    x = np.asarray(inputs["x"], dtype=np.float32)
    w_qkv = np.asarray(inputs["w_qkv"], dtype=np.float32)
    w_proj = np.asarray(inputs["w_proj"], dtype=np.float32)
    b_proj = np.asarray(inputs["b_proj"], dtype=np.float32)

    nc = _get_prog()
    in_maps = _shard(x, w_qkv, w_proj)
    res = run_bass_kernel_spmd(nc, in_maps, core_ids=list(range(NCORES)), **kwargs)

    out = np.zeros((B, T, E), dtype=np.float32)
    for core in range(NCORES):
        out[core // HPC] += np.asarray(res.results[core]["out"], dtype=np.float32)
    out += b_proj[None, None, :]
    return out, res


def kernel(**inputs):
    out, _ = _run(inputs)
    return out


# revision 47
# speedup vs baseline: 1.0069x; 1.0069x over previous
import sys

sys.path.insert(0, "/opt/trn_rl_repo")

import numpy as np
import ml_dtypes

import concourse.bacc as bacc
import concourse.tile as tile
from concourse import mybir
from concourse.bass_utils import run_bass_kernel_spmd

F32 = mybir.dt.float32
BF16 = mybir.dt.bfloat16
Exp = mybir.ActivationFunctionType.Exp

B, T, E = 2, 2048, 1024
H, D = 16, 64
NCORES = 8
HPC = 4
QC = HPC * D
P = 128
KC = E // P
NT = T // P

_PROG = None


def _build():
    nc = bacc.Bacc("TRN2", target_bir_lowering=False, debug=False)

    xt_d = nc.dram_tensor("xt", [E, T], BF16, kind="ExternalInput")
    wqk_d = nc.dram_tensor("wqk", [E, 2 * QC], BF16, kind="ExternalInput")
    wv_d = nc.dram_tensor("wv", [E, QC], BF16, kind="ExternalInput")
    wp_d = nc.dram_tensor("wp", [QC, E], BF16, kind="ExternalInput")
    mk_d = nc.dram_tensor("masks", [P, 2, 256], BF16, kind="ExternalInput")
    out_d = nc.dram_tensor("out", [T, E], BF16, kind="ExternalOutput")

    with tile.TileContext(nc) as tc:
        with (
            tc.tile_pool(name="persist", bufs=1) as persist,
            tc.tile_pool(name="pt", bufs=4) as ptp,
            tc.tile_pool(name="small", bufs=2) as small,
            tc.tile_pool(name="stage", bufs=4) as stg,
            tc.tile_pool(name="spsum", bufs=2, space="PSUM") as spool,
            tc.tile_pool(name="ppsum", bufs=2, space="PSUM") as ppool,
            tc.tile_pool(name="fpsum", bufs=2, space="PSUM") as fpool,
        ):
            qk_sb = [persist.tile([P, T], BF16, name=f"qk{m}") for m in range(4)]
            at_sb = [persist.tile([P, T], BF16, name=f"at{c}") for c in range(2)]
            v_sb = [persist.tile([P, HPC, D + 1], BF16, name=f"v{t}") for t in range(NT)]
            mask_sb = persist.tile([P, 2, 256], BF16, name="masks")
            wp_all = persist.tile([P, 2, E], BF16, name="wp_all")
            xt_all = persist.tile([P, KC, T], BF16, name="xt_all")
            wqk_p = [persist.tile([P, KC, QC], BF16, name=f"wqk_p{i}") for i in range(2)]
            wv_all = persist.tile([P, KC, QC], BF16, name="wv_all")

            xt_r = xt_d.rearrange("(c p) t -> p c t", c=KC)
            wqk_r = wqk_d.rearrange("(c p) w -> p c w", c=KC)
            wv_r = wv_d.rearrange("(c p) w -> p c w", c=KC)
            wp_r = wp_d.rearrange("(c p) e -> p c e", c=2)
            nc.sync.dma_start(out=xt_all[:, :, 0:256], in_=xt_r[:, :, 0:256])
            nc.sync.dma_start(out=wqk_p[0], in_=wqk_r[:, :, 0:256])
            nc.sync.dma_start(out=xt_all[:, :, 256:512], in_=xt_r[:, :, 256:512])
            nc.sync.dma_start(out=mask_sb, in_=mk_d[:])
            nc.sync.dma_start(out=wv_all, in_=wv_r)
            nc.sync.dma_start(out=xt_all[:, :, 512:1024], in_=xt_r[:, :, 512:1024])
            nc.sync.dma_start(out=wqk_p[1], in_=wqk_r[:, :, 256:512])
            nc.sync.dma_start(out=xt_all[:, :, 1024:1536], in_=xt_r[:, :, 1024:1536])
            nc.sync.dma_start(out=xt_all[:, :, 1536:2048], in_=xt_r[:, :, 1536:2048])
            nc.sync.dma_start(out=wp_all, in_=wp_r)

            ones4 = persist.tile([P, HPC, 1], F32, name="ones4")
            nc.vector.memset(ones4, 1.0)
            for t in range(NT):
                nc.vector.tensor_copy(v_sb[t][:, :, D : D + 1], ones4)

            st_tiles = {}

            def qk_chain(mb, u, nl, col0=0, width=512, act_copy=False):
                fp = fpool.tile([P, 512], F32, name="fq", tag="fp")
                base = u * 1024 + nl * 512 + col0
                for c in range(KC):
                    def mm(c=c, fp=fp):
                        nc.tensor.matmul(
                            fp[:, 0:width],
                            lhsT=wqk_p[mb // 2][:, c, (mb % 2) * P : (mb % 2) * P + P],
                            rhs=xt_all[:, c, base : base + width],
                            start=(c == 0),
                            stop=(c == KC - 1),
                        )
                    yield width, mm
                if act_copy:
                    nc.scalar.copy(qk_sb[mb][:, base : base + width], fp[:, 0:width])
                else:
                    nc.vector.tensor_copy(
                        qk_sb[mb][:, base : base + width], fp[:, 0:width]
                    )

            def v_chain(t, act_copy=False):
                fp = fpool.tile([P, 512], F32, name="fv", tag="fp")
                for c in range(KC):
                    def mm(c=c, fp=fp):
                        nc.tensor.matmul(
                            fp[:, 0:QC],
                            lhsT=xt_all[:, c, t * P : (t + 1) * P],
                            rhs=wv_all[:, c, :],
                            start=(c == 0),
                            stop=(c == KC - 1),
                        )
                    yield 256, mm
                dst = v_sb[t][:, :, 0:D]
                srcv = fp[:, 0:QC].rearrange("p (h d) -> p h d", h=HPC)
                if act_copy:
                    nc.scalar.copy(dst, srcv)
                else:
                    nc.vector.tensor_copy(dst, srcv)

            def out_chain(t, nl):
                fp = fpool.tile([P, 512], F32, name="fo", tag="fp")
                for c in range(2):
                    def mm(c=c, fp=fp):
                        nc.tensor.matmul(
                            fp,
                            lhsT=at_sb[c][:, t * P : (t + 1) * P],
                            rhs=wp_all[:, c, nl * 512 : (nl + 1) * 512],
                            start=(c == 0),
                            stop=(c == 1),
                        )
                    yield 512, mm
                tp = t // 2
                if tp not in st_tiles:
                    st_tiles[tp] = stg.tile([P, 2, E], BF16, name=f"st{tp}", tag="st")
                st = st_tiles[tp]
                nc.vector.tensor_copy(st[:, t % 2, nl * 512 : (nl + 1) * 512], fp)
                if nl == 1 and t % 2 == 1:
                    nc.sync.dma_start(
                        out=out_d[(t - 1) * P : (t + 1) * P, :].rearrange(
                            "(s p) e -> p s e", s=2
                        ),
                        in_=st,
                    )

            specs = []
            specs += [("qk000a", qk_chain(0, 0, 0, 0, 256)),
                      ("qk100a", qk_chain(1, 0, 0, 0, 256)),
                      ("qk000b", qk_chain(0, 0, 0, 256, 256)),
                      ("qk100b", qk_chain(1, 0, 0, 256, 256))]
            specs += [("v0", v_chain(0, act_copy=True)), ("v1", v_chain(1, act_copy=True)), ("v2", v_chain(2)), ("v3", v_chain(3))]
            specs += [(f"qk{mb}01", qk_chain(mb, 0, 1, act_copy=True)) for mb in (0, 1)]
            specs += [(f"v{t}", v_chain(t)) for t in range(4, 8)]
            specs += [(f"qk{mb}00", qk_chain(mb, 0, 0, act_copy=True)) for mb in (2, 3)]
            specs += [(f"qk{mb}01", qk_chain(mb, 0, 1, act_copy=True)) for mb in (2, 3)]
            specs += [("qk010", qk_chain(0, 1, 0, act_copy=True)), ("qk110", qk_chain(1, 1, 0, act_copy=True))]
            specs += [(f"out{t}.{nl}", out_chain(t, nl)) for t in (0, 1) for nl in (0, 1)]
            specs += [("v8", v_chain(8, act_copy=True)), ("v9", v_chain(9, act_copy=True))]
            specs += [("qk210", qk_chain(2, 1, 0)), ("qk310", qk_chain(3, 1, 0))]
            specs += [(f"out{t}.{nl}", out_chain(t, nl)) for t in (2, 3) for nl in (0, 1)]
            specs += [("v10", v_chain(10)), ("v11", v_chain(11))]
            specs += [("qk011", qk_chain(0, 1, 1, act_copy=True)), ("qk111", qk_chain(1, 1, 1, act_copy=True))]
            specs += [(f"out{t}.{nl}", out_chain(t, nl)) for t in (4, 5) for nl in (0, 1)]
            specs += [("v12", v_chain(12)), ("v13", v_chain(13))]
            specs += [("qk211", qk_chain(2, 1, 1)), ("qk311", qk_chain(3, 1, 1))]
            specs += [(f"out{t}.{nl}", out_chain(t, nl)) for t in (6, 7) for nl in (0, 1)]
            specs += [("v14", v_chain(14)), ("v15", v_chain(15))]
            specs += [(f"out{t}.{nl}", out_chain(t, nl)) for t in range(8, 16) for nl in (0, 1)]

            names = [n for n, _ in specs]
            assert len(set(names)) == len(names)
            name_idx = {n: i for i, n in enumerate(names)}
            qpos = [0]
            done_names = set()

            def pull(budget_cols, limit=None):
                lim = len(specs) - 1 if limit is None else name_idx[limit]
                spent = 0
                while qpos[0] <= lim and spent < budget_cols:
                    name, g = specs[qpos[0]]
                    try:
                        cost, emit = next(g)
                    except StopIteration:
                        done_names.add(name)
                        qpos[0] += 1
                        continue
                    emit()
                    spent += cost
                return spent

            def drain_through(name):
                while name not in done_names:
                    if qpos[0] >= len(specs):
                        raise RuntimeError(f"drain_through({name}): queue empty")
                    pull(1 << 30, limit=name)

            pend = [None]

            def flush_pend():
                if pend[0] is None:
                    return
                ctx, g, pt, ws = pend[0]
                pend[0] = None
                pso, po, ilast = ctx["pso"], ctx["po"], ctx["ilast"]
                for s in range(2):
                    i = 2 * g + s
                    for hh in range(2):
                        nc.tensor.matmul(
                            pso[:, hh, ws[s] : 256],
                            lhsT=v_sb[i][:, 2 * po + hh, :],
                            rhs=pt[:, hh, s, ws[s] : 256],
                            start=(i == 0 and hh == 0),
                            stop=(i == ilast and hh == 1),
                        )
                if g * 2 + 1 == ilast:
                    q0 = ctx["q0"]
                    den = small.tile([1, 2, 256], F32, name="den", tag="den")
                    nc.vector.tensor_copy(den, pso[D : D + 1, :, 0:256])
                    rec = small.tile([1, 2, 256], F32, name="rec", tag="rec")
                    nc.vector.reciprocal_approx_fast(out=rec, in_=den)
                    rb = small.tile([D, 2, 256], F32, name="rb", tag="rb")
                    nc.gpsimd.partition_broadcast(rb, rec)
                    for hh in range(2):
                        nc.vector.tensor_mul(
                            at_sb[po][64 * hh : 64 * hh + 64, q0 : q0 + 256],
                            pso[0:D, hh, 0:256],
                            rb[:, hh, :],
                        )

            def attn_chain(po, p, jj2, deadlines=None, budget=1024, fence=None):
                deadlines = deadlines or {}
                Qh, Kh = qk_sb[2 * po], qk_sb[2 * po + 1]
                q0 = p * 1024 + jj2 * 256
                ilast = 8 * p + 2 * jj2 + 1
                ngran = (ilast + 1) // 2
                pso = ppool.tile([D + 1, 2, 256], F32, name="pso", tag="pso")
                ctx = {"pso": pso, "po": po, "ilast": ilast, "q0": q0}

                for g in range(ngran):
                    for nm in deadlines.get(g, ()):
                        drain_through(nm)
                    ps = spool.tile([P, 2, 2, 256], F32, name="ps", tag="ps")
                    pt = ptp.tile([P, 2, 2, 256], BF16, name="pt")
                    ws = []
                    for s in range(2):
                        i = 2 * g + s
                        m = i - 8 * p - 2 * jj2
                        w = max(0, 128 * m)
                        ws.append(w)
                        for hh in range(2):
                            nc.tensor.matmul(
                                ps[:, hh, s, w:256],
                                lhsT=Kh[64 * hh : 64 * hh + 64, i * P : (i + 1) * P],
                                rhs=Qh[64 * hh : 64 * hh + 64, q0 + w : q0 + 256],
                                start=(s == 0),
                                stop=(s == 1),
                            )
                    if ws[1] == 0:
                        nc.scalar.activation(pt, ps, Exp, scale=0.125)
                    else:
                        nc.scalar.activation(pt[:, :, 0], ps[:, :, 0], Exp, scale=0.125)
                        nc.scalar.activation(
                            pt[:, :, 1, 128:256], ps[:, :, 1, 128:256],
                            Exp, scale=0.125,
                        )
                    flush_pend()
                    pull(budget, limit=fence)
                    for s in range(2):
                        m = 2 * g + s - 8 * p - 2 * jj2
                        if m >= 0:
                            for hh in range(2):
                                nc.vector.tensor_mul(
                                    pt[:, hh, s, ws[s] : 256],
                                    pt[:, hh, s, ws[s] : 256],
                                    mask_sb[:, m, ws[s] : 256],
                                )
                    pend[0] = (ctx, g, pt, ws)
                for nm in deadlines.get(ngran, ()):
                    drain_through(nm)

            drain_through("qk100a")

            attn_chain(0, 0, 0, fence="qk111", budget=1280,
                       deadlines={1: ["qk100b", "v0", "v1"]})
            attn_chain(0, 0, 1, fence="qk111", budget=1280,
                       deadlines={1: ["qk001"], 2: ["v2", "v3"]})
            attn_chain(0, 0, 2, fence="qk111", budget=1280,
                       deadlines={1: ["qk101"], 3: ["v4", "v5"]})
            attn_chain(0, 0, 3, fence="qk111", budget=1280,
                       deadlines={3: ["qk200", "qk300"], 4: ["v6", "v7"]})
            attn_chain(1, 0, 0, fence="qk111", budget=1280, deadlines={})
            attn_chain(1, 0, 1, fence="out1.1", budget=1280,
                       deadlines={1: ["qk201"]})
            attn_chain(1, 0, 2, fence="out3.1", budget=1280,
                       deadlines={1: ["qk301"]})
            attn_chain(1, 0, 3, fence="out3.1", budget=1280,
                       deadlines={3: ["qk010"]})
            attn_chain(0, 1, 0, fence="out3.1",
                       deadlines={3: ["qk110"], 4: ["qk210"], 5: ["v8", "v9"]})
            attn_chain(1, 1, 0, fence="out5.1",
                       deadlines={3: ["qk310"], 4: ["qk011"]})
            attn_chain(0, 1, 1, fence="out9.1",
                       deadlines={5: ["qk111"], 6: ["v10", "v11"]})
            attn_chain(1, 1, 1, fence="out9.1",
                       deadlines={5: ["qk211"]})
            attn_chain(0, 1, 2, fence="out11.1",
                       deadlines={6: ["qk311"], 7: ["v12", "v13"]})
            attn_chain(1, 1, 2, fence="out11.1", deadlines={})
            attn_chain(0, 1, 3, fence="out13.1",
                       deadlines={8: ["v14", "v15"]})
            attn_chain(1, 1, 3, fence="out13.1", deadlines={})
            flush_pend()

            while qpos[0] < len(specs):
                pull(1 << 30)

    nc.compile()
    return nc


def _get_prog():
    global _PROG
    if _PROG is None:
        _PROG = _build()
    return _PROG


def _masks_np():
    kk = np.arange(P)[:, None]
    qq = np.arange(256)[None, :]
    return np.stack(
        [((128 * m + kk) <= qq) for m in range(2)], axis=1
    ).astype(ml_dtypes.bfloat16)


def _bf(a):
    return np.ascontiguousarray(a).astype(ml_dtypes.bfloat16)


def _shard(x, w_qkv, w_proj):
    masks = _masks_np()
    in_maps = []
    for core in range(NCORES):
        b, g = core // HPC, core % HPC
        c0 = g * QC
        wq = w_qkv[:, c0 : c0 + QC]
        wk = w_qkv[:, E + c0 : E + c0 + QC]
        wqk = np.concatenate(
            [wq[:, 0:128], wk[:, 0:128], wq[:, 128:256], wk[:, 128:256]], axis=1
        )
        in_maps.append(
            {
                "xt": _bf(x[b].T),
                "wqk": _bf(wqk),
                "wv": _bf(w_qkv[:, 2 * E + c0 : 2 * E + c0 + QC]),
                "wp": _bf(w_proj[c0 : c0 + QC, :]),
                "masks": masks,
            }
        )
    return in_maps


def _run(inputs, **kwargs):
    x = np.asarray(inputs["x"], dtype=np.float32)
    w_qkv = np.asarray(inputs["w_qkv"], dtype=np.float32)
    w_proj = np.asarray(inputs["w_proj"], dtype=np.float32)
    b_proj = np.asarray(inputs["b_proj"], dtype=np.float32)

    nc = _get_prog()
    in_maps = _shard(x, w_qkv, w_proj)
    res = run_bass_kernel_spmd(nc, in_maps, core_ids=list(range(NCORES)), **kwargs)

    out = np.zeros((B, T, E), dtype=np.float32)
    for core in range(NCORES):
        out[core // HPC] += np.asarray(res.results[core]["out"], dtype=np.float32)
    out += b_proj[None, None, :]
    return out, res


def kernel(**inputs):
    out, _ = _run(inputs)
    return out
